# revision 90
# baseline (speedup 1.0000x reference)
"""Trainium2 Bass kernel for nn_EncoderWithClassifier (4-layer encoder + classifier).

Sharding: 8 cores, core c handles (batch b=c//2, sequence half th=c%2, 1024 tokens).
Canonical activation layout: x^T [C=256 (2 chunks of 128 partitions), T_local=1024],
residual stream fp32 with a bf16 shadow for LN statistics; all matmuls run on bf16
operands (1 PE cycle/row vs 4 for fp32).

Attention: scores S [s_tile=128, t] from row-packed K=32 matmuls (4 heads via
tile_position), one exp per (2-head group, s_tile) [128,1024] PSUM->SBUF (bf16).
o is accumulated output-transposed: out [t_chunk=128, 33] per (head, t_chunk),
lhsT = exp[s,t] chunk, rhs = v_ext[s, 33] whose last column is ones -- so the
softmax denominator accumulates in column 32 for free. Normalize on DVE with
per-partition (per-token) reciprocal scalars, transpose back to [c,t] on the PE.

LayerNorm: mean/mean-square are computed directly REPLICATED across all 128
partitions via matmuls with an all-1/256 [128,128] lhsT (same cost as a [1,N]
output), so all stat math runs at full DVE parallelism; rstd = exp(-0.5 ln(var))
keeps every activation in one act-function table (no table reloads).

Schedule: a software pipeline keyed to keeping the Activation engine (the
bottleneck: 512 exp instructions at ~1.04us) saturated. Per layer, the tcn0
windows run in three s-phases (w0-local for both head groups, then w1-local,
then remote) with the partial o-accumulator spilled to SBUF between phases,
so 16+ exps are always in flight while the FFN-tail -> LN1(w1) -> collective
chain resolves. Non-attention work (epilogues, proj/FFN, next-layer LN1(w0) +
local Q/K/V, final-LN/pool) is interleaved between s-tiles as filler thunks.

Per layer one 2-rank AllGather exchanges the LN1 output (bf16) between sequence
halves; remote half lands via dma_gather with host-provided row indices (keeps
the SPMD program rank-symmetric). The collective is issued right after LN1 and
only the remote-half K/V matmuls wait on it. Final mean-pool partials are
exchanged with an AllGather and summed locally.

PSUM budget (8 banks): S 2x[128,1024] = 4, o_acc 2x[128,264 pad 512] = 2,
mm 2x[128,512] = 2.
"""
import numpy as np
import ml_dtypes

import concourse.bacc as bacc
import concourse.mybir as mybir
import concourse.tile as tile
from concourse import bass_utils, library_config
from concourse.masks import make_identity

V, C, TMAX, H, L = 32000, 256, 2048, 8, 4
HS, FFN = 32, 256
CLS_H, NOUT = 512, 10
B, T = 4, 2048
TL = 1024          # tokens per core
P = 128
EPS = 1e-5
SCALE = C ** (-0.5)
N_CORES = 8
dt = mybir.dt
F32 = dt.float32
BF16 = dt.bfloat16
NPBF16 = ml_dtypes.bfloat16
Alu = mybir.AluOpType
Act = mybir.ActivationFunctionType
X_AXIS = mybir.AxisListType.X

_CACHE = {}


class _Bacc(bacc.Bacc):
    def insert_act_table_loads(self):
        """Same pass as the base class, but with Exp/Ln stripped from every
        activation-function set except the combined natural_log_exp set, so
        the greedy table assignment lands all Ln and Exp activations in ONE
        table (set ids / real-HW semantics unchanged) instead of ping-ponging
        between 'exp_and_others' and 'natural_log' with a 1.3us table load at
        every switch."""
        import bass_rust as _br
        import concourse.mybir as _mb
        from concourse.hw_specs import get_activation_tables
        has_activation = any(
            isinstance(i, _mb.InstActivation)
            for b in self.main_func.blocks
            for i in b.instructions
        )
        if not has_activation:
            return
        tabs = list(get_activation_tables(self.m.arch).items())
        filt = []
        for name, s in tabs:
            if name == "natural_log_exp_and_others":
                filt.append((name, s))
            else:
                filt.append((name, {f for f in s
                                    if f.name not in ("Exp", "Ln")}))
        _br.insert_act_table_loads(self, filt)


def _build_program(sim=False):
    nc = _Bacc("TRN2", target_bir_lowering=False, debug=False,
               num_devices=1 if sim else N_CORES)

    # ---------------- dram I/O ----------------
    tok = nc.dram_tensor("tok", [V, C], F32, kind="ExternalInput")
    idxw = nc.dram_tensor("idxw", [P, TL // 16], dt.int16, kind="ExternalInput")
    posr = nc.dram_tensor("posr", [P, TL // P, C], F32, kind="ExternalInput")
    remidx = nc.dram_tensor("remidx", [P, (2 * P) // 16], dt.int16,
                            kind="ExternalInput")
    wq_d = nc.dram_tensor("wq", [L, P, 2, C], BF16, kind="ExternalInput")
    wk_d = nc.dram_tensor("wk", [L, P, 2, C], BF16, kind="ExternalInput")
    wv_d = nc.dram_tensor("wv", [L, P, 2, C], BF16, kind="ExternalInput")
    wp_d = nc.dram_tensor("wp", [L, P, 2, C], BF16, kind="ExternalInput")
    w1_d = nc.dram_tensor("w1", [L, P, 2, FFN], BF16, kind="ExternalInput")
    w2_d = nc.dram_tensor("w2", [L, P, 2, C], BF16, kind="ExternalInput")
    vecs_d = nc.dram_tensor("vecs", [L, P, 7, 2], F32, kind="ExternalInput")
    # vecs order: ln1_g, ln1_b, ln2_g, ln2_b, bproj, b1, b2
    lnf_d = nc.dram_tensor("lnf", [P, 2, 2], F32, kind="ExternalInput")   # g, b
    wc1_d = nc.dram_tensor("wc1", [P, 2, CLS_H], F32, kind="ExternalInput")
    bc1_d = nc.dram_tensor("bc1", [P, CLS_H // P], F32, kind="ExternalInput")
    wc2_d = nc.dram_tensor("wc2", [P, CLS_H // P, NOUT], F32, kind="ExternalInput")
    bc2_d = nc.dram_tensor("bc2", [1, NOUT], F32, kind="ExternalInput")
    out_d = nc.dram_tensor("probs", [1, NOUT], F32, kind="ExternalOutput")

    REPL = [[0, 1], [2, 3], [4, 5], [6, 7]]

    with tile.TileContext(nc) as tc:
        with (
            tc.tile_pool(name="const", bufs=1) as cp,
            tc.tile_pool(name="work", bufs=1) as wk,
            tc.tile_pool(name="exp", bufs=12) as ep,
            tc.tile_pool(name="small", bufs=2) as sp,
            tc.tile_pool(name="psS", bufs=2, space="PSUM") as psS,
            tc.tile_pool(name="psO", bufs=1, space="PSUM") as psO,
            tc.tile_pool(name="psM", bufs=2, space="PSUM") as psM,
            tc.tile_pool(name="dram", bufs=3, space="DRAM") as dp,
        ):
            nc.gpsimd.load_library(library_config.mlp)

            # ---------------- constants / weights to SBUF ----------------
            ident = cp.tile([P, P], F32, tag="ident")
            make_identity(nc, ident[:])
            inv256R = cp.tile([P, P], BF16, tag="inv256R")
            nc.vector.memset(inv256R[:], 1.0 / C)

            def load_const(name, dram_ap, shape, dtype=F32):
                t = cp.tile(shape, dtype, tag=name, name=name)
                nc.sync.dma_start(t[:], dram_ap)
                return t

            # DMA emission order = arrival order: gather indices first (the
            # embedding gather only needs those), then per-layer weights in
            # first-use order so compute starts while later layers stream in.
            idx_sb = load_const("idx_sb", idxw[:], [P, TL // 16], dt.int16)
            remidx_sb = load_const("remidx_sb", remidx[:], [P, (2 * P) // 16],
                                   dt.int16)
            vecs = [load_const(f"vec{l}", vecs_d[l], [P, 7, 2]) for l in range(L)]

            # vecs[l] rows: 0 ln1_g, 1 ln1_b, 2 ln2_g, 3 ln2_b, 4 bproj, 5 b1, 6 b2
            def vap(l, row, cc):
                return vecs[l][:, row, cc:cc + 1]

            # persistent activations
            xT = [wk.tile([P, TL], F32, tag=f"xT{cc}", name=f"xT{cc}")
                  for cc in range(2)]
            xbf = [wk.tile([P, TL], BF16, tag=f"xbf{cc}", name=f"xbf{cc}")
                   for cc in range(2)]
            # ---------------- embedding ----------------
            with tc.tile_pool(name="embed", bufs=1) as ebp:
                xg = ebp.tile([P, TL // P, C], F32, tag="xg")
                # gather in halves: LN1(w0) only needs the first 512 tokens,
                # so the embedding front half starts ~2us earlier
                nc.gpsimd.dma_gather(xg[:, 0:4, :], tok[:],
                                     idx_sb[:, 0:TL // 32], 512, 512, C)
                nc.gpsimd.dma_gather(xg[:, 4:8, :], tok[:],
                                     idx_sb[:, TL // 32:], 512, 512, C)
                pos_sb = ebp.tile([P, TL // P, C], F32, tag="pos_sb")
                nc.sync.dma_start(pos_sb[:], posr[:])
                for tt in range(TL // P):
                    nc.vector.tensor_add(xg[:, tt, :], xg[:, tt, :],
                                         pos_sb[:, tt, :])
                    for cc in range(2):
                        tp = psM.tile([P, P], F32, tag="mm", name="tp",
                                      padded_shape=[P, 512])
                        nc.tensor.transpose(tp[:], xg[:, tt, cc * P:(cc + 1) * P],
                                            ident[:])
                        sl = slice(tt * P, (tt + 1) * P)
                        # Act is idle at startup: drain half the transposes
                        # there and make the bf16 shadow on Act as well.
                        if tt % 2 == 0:
                            nc.vector.tensor_copy(xT[cc][:, sl], tp[:])
                        else:
                            nc.scalar.copy(xT[cc][:, sl], tp[:])
                        nc.scalar.copy(xbf[cc][:, sl], xT[cc][:, sl])

            wq, wkt, wv, wp, w1, w2 = [], [], [], [], [], []
            for l in range(L):
                wq.append(load_const(f"wq{l}", wq_d[l], [P, 2, C], BF16))
                wkt.append(load_const(f"wk{l}", wk_d[l], [P, 2, C], BF16))
                wv.append(load_const(f"wv{l}", wv_d[l], [P, 2, C], BF16))
                wp.append(load_const(f"wp{l}", wp_d[l], [P, 2, C], BF16))
                w1.append(load_const(f"w1{l}", w1_d[l], [P, 2, FFN], BF16))
                w2.append(load_const(f"w2{l}", w2_d[l], [P, 2, C], BF16))
            lnf = load_const("lnf", lnf_d[:], [P, 2, 2])
            wc1 = load_const("wc1", wc1_d[:], [P, 2, CLS_H])
            bc1 = load_const("bc1", bc1_d[:], [P, CLS_H // P])
            wc2 = load_const("wc2", wc2_d[:], [P, CLS_H // P, NOUT])
            bc2 = load_const("bc2", bc2_d[:], [1, NOUT])

            # ---------------- layernorm helper (replicated stats) ----------
            # Processes one 512-token window `nch` of LN(x) into out[cc][:, sl].
            # crit=True routes mu^2 through the (idle-at-that-point) Act
            # engine, shortening the serial DVE chain at layer boundaries.
            def ln_window(out, nch, g_of, b_of, crit=False):
                sl = slice(nch * 512, (nch + 1) * 512)
                xsq = sp.tile([P, 512], BF16, tag="lnsq", name="lnsq")
                nc.vector.tensor_mul(xsq[:], xbf[0][:, sl], xbf[0][:, sl])
                xsq2 = sp.tile([P, 512], BF16, tag="lnsq2", name="lnsq2")
                nc.vector.tensor_mul(xsq2[:], xbf[1][:, sl], xbf[1][:, sl])
                muR = psM.tile([P, 512], F32, tag="mm", name="muR")
                nc.tensor.matmul(muR[:], lhsT=inv256R[:], rhs=xbf[0][:, sl],
                                 start=True, stop=False)
                nc.tensor.matmul(muR[:], lhsT=inv256R[:], rhs=xbf[1][:, sl],
                                 start=False, stop=True)
                msqR = psM.tile([P, 512], F32, tag="mm", name="msqR")
                nc.tensor.matmul(msqR[:], lhsT=inv256R[:], rhs=xsq[:],
                                 start=True, stop=False)
                nc.tensor.matmul(msqR[:], lhsT=inv256R[:], rhs=xsq2[:],
                                 start=False, stop=True)
                # HW: an op may read at most ONE non-scalar input from PSUM,
                # so land mu^2 in SBUF before the variance op.
                musq = sp.tile([P, 512], F32, tag="musq", name="musq")
                mu_sb = sp.tile([P, 512], F32, tag="mu_sb", name="mu_sb")
                if crit:
                    nc.scalar.activation(musq[:], muR[:], Act.Square)
                    nc.scalar.copy(mu_sb[:], muR[:])
                else:
                    nc.vector.tensor_copy(mu_sb[:], muR[:])
                    nc.vector.tensor_mul(musq[:], mu_sb[:], mu_sb[:])
                varb = sp.tile([P, 512], F32, tag="varb", name="varb")
                nc.vector.scalar_tensor_tensor(varb[:], msqR[:], EPS, musq[:],
                                               Alu.add, Alu.subtract)
                # rstd = exp(-0.5*ln(var)); Ln+Exp live in one act table with
                # the attention Exp, so no ACT_TABLE_LOAD is ever issued.
                stdb = sp.tile([P, 512], F32, tag="stdb", name="stdb")
                nc.scalar.activation(stdb[:], varb[:], Act.Ln)
                rstd = sp.tile([P, 512], F32, tag="rstd", name="rstd")
                nc.scalar.activation(rstd[:], stdb[:], Act.Exp, scale=-0.5)
                mrs = sp.tile([P, 512], F32, tag="mrs", name="mrs")
                nc.vector.tensor_mul(mrs[:], mu_sb[:], rstd[:])
                for cc in range(2):
                    t1 = sp.tile([P, 512], F32, tag=f"lnt{cc}", name=f"lnt{cc}")
                    nc.vector.tensor_mul(t1[:], xT[cc][:, sl], rstd[:])
                    nc.vector.tensor_sub(t1[:], t1[:], mrs[:])
                    nc.vector.tensor_scalar(out[cc][:, sl], t1[:],
                                            g_of(cc), b_of(cc), Alu.mult, Alu.add)

            # ---------------- transformer layers ----------------
            # Attention-side tiles are double-buffered by layer parity so each
            # layer's front (LN1 w0 + local-w0 Q/K/V) can be emitted during
            # the previous layer's last attention window without WAR stalls.
            hTp = {p: [wk.tile([P, TL], BF16, tag=f"hT{p}{cc}",
                               name=f"hT{p}{cc}") for cc in range(2)]
                   for p in range(2)}
            qTp = {p: [wk.tile([P, TL], BF16, tag=f"qT{p}{mt}",
                               name=f"qT{p}{mt}") for mt in range(2)]
                   for p in range(2)}
            kTp = {p: [wk.tile([P, T], BF16, tag=f"kT{p}{mt}",
                               name=f"kT{p}{mt}") for mt in range(2)]
                   for p in range(2)}
            # [p, s-half, kc, 512]: each 512-token gather half is contiguous
            hRp = {p: wk.tile([P, 2, 2, 512], BF16, tag=f"hR{p}", name=f"hR{p}")
                   for p in range(2)}
            v_sbp = {p: [wk.tile([P, H, HS + 1], BF16, tag=f"v{p}_{st}",
                                 name=f"v{p}_{st}") for st in range(16)]
                     for p in range(2)}
            for p in range(2):
                for st in range(16):
                    nc.vector.memset(v_sbp[p][st][:, :, HS:HS + 1], 1.0)
            h2T = [wk.tile([P, TL], BF16, tag=f"h2T{cc}", name=f"h2T{cc}")
                   for cc in range(2)]
            oT = [wk.tile([P, TL], BF16, tag=f"oT{cc}", name=f"oT{cc}")
                  for cc in range(2)]
            fT = [wk.tile([P, TL], BF16, tag=f"fT{ff}", name=f"fT{ff}")
                  for ff in range(2)]

            W = HS + 1

            def q_mats(l, w, on_act=False):
                par = l % 2
                hT, qT = hTp[par], qTp[par]
                sl = slice(w * 512, (w + 1) * 512)
                for mt in range(2):
                    qps = psM.tile([P, 512], F32, tag="mm", name="qps")
                    for kc in range(2):
                        nc.tensor.matmul(qps[:],
                                         lhsT=wq[l][:, kc, mt * P:(mt + 1) * P],
                                         rhs=hT[kc][:, sl],
                                         start=(kc == 0), stop=(kc == 1))
                    if on_act:
                        nc.scalar.copy(qT[mt][:, sl], qps[:])
                    else:
                        nc.vector.tensor_copy(qT[mt][:, sl], qps[:])

            def kv_local(l, w, on_act=False):
                par = l % 2
                hT, kT, v_sb = hTp[par], kTp[par], v_sbp[par]
                sl = slice(w * 512, (w + 1) * 512)
                for mt in range(2):
                    kps = psM.tile([P, 512], F32, tag="mm", name="kps")
                    for kc in range(2):
                        nc.tensor.matmul(kps[:],
                                         lhsT=wkt[l][:, kc, mt * P:(mt + 1) * P],
                                         rhs=hT[kc][:, sl],
                                         start=(kc == 0), stop=(kc == 1))
                    if on_act:
                        nc.scalar.copy(kT[mt][:, sl], kps[:])
                    else:
                        nc.vector.tensor_copy(kT[mt][:, sl], kps[:])
                for st in range(4 * w, 4 * w + 4):
                    vps = psM.tile([P, H, HS], F32, tag="mm", name="vps")
                    for kc in range(2):
                        nc.tensor.matmul(vps[:],
                                         lhsT=hT[kc][:, st * P:(st + 1) * P],
                                         rhs=wv[l][:, kc, :],
                                         start=(kc == 0), stop=(kc == 1))
                    if on_act:
                        nc.scalar.copy(v_sb[st][:, :, 0:HS], vps[:])
                    else:
                        nc.vector.tensor_copy(v_sb[st][:, :, 0:HS], vps[:])

            def kv_remote(l):
                par = l % 2
                hR, kT, v_sb = hRp[par], kTp[par], v_sbp[par]
                for mt in range(2):
                    for nch in range(2):
                        sl = slice(1024 + nch * 512, 1024 + (nch + 1) * 512)
                        rsl = slice(nch * 512, (nch + 1) * 512)
                        kps = psM.tile([P, 512], F32, tag="mm", name="kpr")
                        for kc in range(2):
                            nc.tensor.matmul(
                                kps[:],
                                lhsT=wkt[l][:, kc, mt * P:(mt + 1) * P],
                                rhs=hR[:, kc, rsl],
                                start=(kc == 0), stop=(kc == 1))
                        nc.vector.tensor_copy(kT[mt][:, sl], kps[:])
                for st in range(8, 16):
                    vps = psM.tile([P, H, HS], F32, tag="mm", name="vpr")
                    for kc in range(2):
                        nc.tensor.matmul(
                            vps[:],
                            lhsT=hR[:, kc, (st - 8) * P:(st - 7) * P],
                            rhs=wv[l][:, kc, :],
                            start=(kc == 0), stop=(kc == 1))
                    nc.vector.tensor_copy(v_sb[st][:, :, 0:HS], vps[:])

            def attn_sts(l, tcn, hp, oacc, sts, first, last, fillers=(),
                         lag=2):
                """Emit S/exp for each s-tile, with the o-matmuls emitted
                `lag` s-tiles behind: an o-matmul whose dependency (exp, or
                the o-accumulator's WAR on a spill) is unresolved parks in
                the PE's depth-4 wait queue and blocks every S matmul behind
                it, stalling the exp stream. With the lag, its inputs are
                always long since resolved. After each s-tile one filler
                thunk (epilogues / proj+FFN / next-layer prologue pieces) is
                emitted so tail work interleaves with the exp stream."""
                par = l % 2
                qT, kT, v_sb = qTp[par], kTp[par], v_sbp[par]
                tsl = slice(tcn * 512, (tcn + 1) * 512)
                fillers = list(fillers)
                pend = []

                def emit_o(st, expg):
                    for g in range(2):
                        for jj in range(2):
                            j = 2 * g + jj
                            hg = hp * 4 + j
                            for ct in range(4):
                                tgt = oacc[ct // 2]
                                nc.tensor.matmul(
                                    tgt[:, ct % 2, j, :],
                                    lhsT=expg[g][:, jj * 512 + ct * P:
                                                 jj * 512 + (ct + 1) * P],
                                    rhs=v_sb[st][:, hg, :],
                                    start=(st == first), stop=(st == last))

                for st in sts:
                    Sg = []
                    for g in range(2):
                        S = psS.tile([P, 1024], F32, tag="S", name="S")
                        for jj in range(2):
                            j = 2 * g + jj
                            nc.tensor.matmul(
                                S[:, jj * 512:(jj + 1) * 512],
                                lhsT=kT[hp][32 * j:32 * (j + 1),
                                            st * P:(st + 1) * P],
                                rhs=qT[hp][32 * j:32 * (j + 1), tsl],
                                start=True, stop=True,
                                tile_position=(32 * j, 0))
                        Sg.append(S)
                    expg = []
                    for g in range(2):
                        expT = ep.tile([P, 1024], BF16, tag="expT",
                                       name="expT")
                        nc.scalar.activation(expT[:], Sg[g][:], Act.Exp,
                                             scale=SCALE)
                        expg.append(expT)
                    pend.append((st, expg))
                    if len(pend) > lag:
                        emit_o(*pend.pop(0))
                    if fillers:
                        fillers.pop(0)()
                for st_, expg_ in pend:
                    emit_o(st_, expg_)
                for f in fillers:
                    f()

            def epilogue(tcn, hp, oacc, part=None):
                # normalize (per-token reciprocal of denominator column)
                # + transpose back to [c, t]; two thunks of 2 t-chunks each.
                # With `part` (spilled local-phase partial), merge it first.
                # oacc=None: `part` is the sole (SBUF) source — used for the
                # last window so its PSUM banks are released by fast Act
                # copies instead of by this DVE-queued epilogue.
                def emit_cts(cts):
                    for ct in cts:
                        half = ct % 2
                        if oacc is None:
                            tgt_h = part[ct // 2][:, half, :, :]
                        elif part is not None:
                            tgt = oacc[ct // 2]
                            m = sp.tile([P, 4, W], F32, tag="omrg", name="omrg")
                            nc.vector.tensor_add(m[:], tgt[:, half, :, :],
                                                 part[ct // 2][:, half, :, :])
                            tgt_h = m[:, :, :]
                        else:
                            tgt = oacc[ct // 2]
                            tgt_h = tgt[:, half, :, :]
                        rec = sp.tile([P, 4, 1], F32, tag="rec", name="rec")
                        nc.vector.reciprocal(rec[:], tgt_h[:, :, HS:HS + 1])
                        onrm = sp.tile([P, 4, HS], F32, tag="onrm", name="onrm")
                        nc.vector.tensor_mul(onrm[:], tgt_h[:, :, 0:HS],
                                             rec[:].broadcast_to([P, 4, HS]))
                        tp = psM.tile([P, P], F32, tag="mm", name="otp",
                                      padded_shape=[P, 512])
                        nc.tensor.transpose(tp[:], onrm[:], ident[:])
                        nc.vector.tensor_copy(
                            oT[hp][:, tcn * 512 + ct * P:
                                   tcn * 512 + (ct + 1) * P], tp[:])
                return [lambda: emit_cts([0, 1]), lambda: emit_cts([2, 3])]

            def spill(hp, oacc, part=None):
                """Copy (or add) the phase-partial o-accumulator to SBUF so
                the PSUM banks can be reused before later K/V are ready. The
                first-phase copy rides the Act engine: at layer fronts the
                DVE queue is saturated with the previous layer's FFN tail,
                and a DVE spill there would stall the next window's
                o-matmuls (and the PE queue behind them)."""
                if part is None:
                    part = [sp.tile([P, 2, 4, W], F32, tag=f"osp{hp}{half}",
                                    name=f"osp{hp}{half}") for half in range(2)]
                    for half in range(2):
                        nc.scalar.copy(part[half][:], oacc[half][:])
                else:
                    for half in range(2):
                        nc.vector.tensor_add(part[half][:], part[half][:],
                                             oacc[half][:])
                return part

            def proj_ffn_thunks(l, tcn):
                tsl = slice(tcn * 512, (tcn + 1) * 512)

                def proj(cc):
                    dpj = psM.tile([P, 512], F32, tag="mm", name="dpj")
                    for kc in range(2):
                        nc.tensor.matmul(dpj[:],
                                         lhsT=wp[l][:, kc, cc * P:(cc + 1) * P],
                                         rhs=oT[kc][:, tsl],
                                         start=(kc == 0), stop=(kc == 1))
                    nc.vector.scalar_tensor_tensor(xT[cc][:, tsl], dpj[:],
                                                   vap(l, 4, cc),
                                                   xT[cc][:, tsl],
                                                   Alu.add, Alu.add)
                    nc.vector.tensor_copy(xbf[cc][:, tsl], xT[cc][:, tsl])

                def ln2():
                    ln_window(h2T, tcn, lambda cc: vap(l, 2, cc),
                              lambda cc: vap(l, 3, cc), crit=(tcn == 1))

                def ffn1():
                    for ff in range(2):
                        fps = psM.tile([P, 512], F32, tag="mm", name="fps")
                        for kc in range(2):
                            nc.tensor.matmul(fps[:],
                                             lhsT=w1[l][:, kc, ff * P:(ff + 1) * P],
                                             rhs=h2T[kc][:, tsl],
                                             start=(kc == 0), stop=(kc == 1))
                        nc.vector.tensor_scalar(fT[ff][:, tsl], fps[:],
                                                vap(l, 5, ff), 0.0,
                                                Alu.add, Alu.max)

                def ffn2():
                    for cc in range(2):
                        d2 = psM.tile([P, 512], F32, tag="mm", name="d2")
                        for kc in range(2):
                            nc.tensor.matmul(d2[:],
                                             lhsT=w2[l][:, kc, cc * P:(cc + 1) * P],
                                             rhs=fT[kc][:, tsl],
                                             start=(kc == 0), stop=(kc == 1))
                        nc.vector.scalar_tensor_tensor(xT[cc][:, tsl], d2[:],
                                                       vap(l, 6, cc),
                                                       xT[cc][:, tsl],
                                                       Alu.add, Alu.add)
                        nc.vector.tensor_copy(xbf[cc][:, tsl], xT[cc][:, tsl])

                return [lambda: proj(0), lambda: proj(1), ln2, ffn1, ffn2]

            def new_oacc():
                # [t=128, ct-half, head, HS+1] padded to a 64-wide head slot
                # so every accumulation region is 64-aligned and each tile is
                # exactly one PSUM bank.
                return [psO.tile([P, 2, 4, W], F32, tag=f"o{half}",
                                 name=f"o{half}", padded_shape=[P, 2, 4, 64])
                        for half in range(2)]

            def prologue_thunks(l, on_act=False):
                """LN1 window-0 + local-w0 Q/K/V + the ENTIRE w0 half of the
                h exchange (collective #1 + gather), as filler thunks
                interleaved into the previous layer's last attention window
                (x[w0] is final once that layer's proj_ffn(0) ran). Shipping
                the w0 half a whole window early means the first half of the
                remote s-tiles never waits on the slow w1 chain."""
                par = l % 2

                b_in0 = dp.tile([2 * P, 512], BF16, tag="b_in0",
                                name="b_in0")

                def ln1w0():
                    ln_window(hTp[par], 0, lambda cc: vap(l, 0, cc),
                              lambda cc: vap(l, 1, cc))
                    for cc in range(2):
                        nc.sync.dma_start(b_in0[cc * P:(cc + 1) * P, :],
                                          hTp[par][cc][:, 0:512])

                def coll0():
                    b_out = dp.tile([4 * P, 512], BF16, tag="b_out0",
                                    name="b_out0")
                    if sim:
                        nc.sync.dma_start(b_out[:2 * P, :], b_in0[:])
                        nc.sync.dma_start(b_out[2 * P:, :], b_in0[:])
                    else:
                        nc.gpsimd.collective_compute(
                            "AllGather", Alu.bypass, replica_groups=REPL,
                            ins=[b_in0[:].opt()], outs=[b_out[:].opt()])
                    nc.gpsimd.dma_gather(hRp[par][:, 0], b_out[:],
                                         remidx_sb[:], 2 * P, 2 * P, 512)

                return [ln1w0,
                        lambda: q_mats(l, 0, on_act=on_act),
                        coll0,
                        lambda: kv_local(l, 0, on_act=on_act)]

            def kv_remote_pieces(l):
                """Remote K/V thunks split by s-half: the `nch=0` pieces only
                need collective #1 (w0 h, shipped a window early)."""
                par = l % 2
                hR, kT, v_sb = hRp[par], kTp[par], v_sbp[par]

                def kpart(mt, nch):
                    sl = slice(1024 + nch * 512, 1024 + (nch + 1) * 512)
                    kps = psM.tile([P, 512], F32, tag="mm", name="kpr")
                    for kc in range(2):
                        nc.tensor.matmul(
                            kps[:],
                            lhsT=wkt[l][:, kc, mt * P:(mt + 1) * P],
                            rhs=hR[:, nch, kc, :],
                            start=(kc == 0), stop=(kc == 1))
                    nc.vector.tensor_copy(kT[mt][:, sl], kps[:])

                def vpart(s0):
                    for st in range(s0, s0 + 4):
                        r = st - 8
                        vps = psM.tile([P, H, HS], F32, tag="mm", name="vpr")
                        for kc in range(2):
                            nc.tensor.matmul(
                                vps[:],
                                lhsT=hR[:, r // 4, kc,
                                        (r % 4) * P:(r % 4 + 1) * P],
                                rhs=wv[l][:, kc, :],
                                start=(kc == 0), stop=(kc == 1))
                        nc.vector.tensor_copy(v_sb[st][:, :, 0:HS], vps[:])

                w0 = [lambda: kpart(0, 0), lambda: kpart(1, 0),
                      lambda: vpart(8)]
                w1 = [lambda: kpart(0, 1), lambda: kpart(1, 1),
                      lambda: vpart(12)]
                return w0, w1

            # final-LN + mean-pool per window (xfT reuses the parity-0 hT
            # tiles, which the last layer doesn't touch)
            xfT = hTp[L % 2]
            emb4 = sp.tile([P, 2, 2], F32, tag="emb4")

            def lnf_pool(w):
                ln_window(xfT, w, lambda cc: lnf[:, 0, cc:cc + 1],
                          lambda cc: lnf[:, 1, cc:cc + 1], crit=(w == 1))
                sl = slice(w * 512, (w + 1) * 512)
                for cc in range(2):
                    nc.vector.reduce_sum(emb4[:, w, cc:cc + 1],
                                         xfT[cc][:, sl], axis=X_AXIS)

            for t in prologue_thunks(0, on_act=True):
                t()
            for l in range(L):
                par = l % 2
                hT, hR = hTp[par], hRp[par]

                # ---- tcn0 in three phases over s: w0-local tiles for BOTH
                # head-groups first (16-exp runway for the LN1(w1) chain),
                # then w1-local (16 more before the collective is needed),
                # then remote; partial o spills to SBUF between phases. ----
                oaccA = new_oacc()
                attn_sts(l, 0, 0, oaccA, range(0, 4), 0, 3)
                part00 = spill(0, oaccA)

                def ln1w1_coll():
                    ln_window(hT, 1, lambda cc: vap(l, 0, cc),
                              lambda cc: vap(l, 1, cc), crit=True)
                    b_in1 = dp.tile([2 * P, 512], BF16, tag="b_in1",
                                    name="b_in1")
                    for cc in range(2):
                        nc.sync.dma_start(b_in1[cc * P:(cc + 1) * P, :],
                                          hT[cc][:, 512:1024])
                    b_out = dp.tile([4 * P, 512], BF16, tag="b_out1",
                                    name="b_out1")
                    if sim:
                        nc.sync.dma_start(b_out[:2 * P, :], b_in1[:])
                        nc.sync.dma_start(b_out[2 * P:, :], b_in1[:])
                    else:
                        nc.gpsimd.collective_compute(
                            "AllGather", Alu.bypass, replica_groups=REPL,
                            ins=[b_in1[:].opt()],
                            outs=[b_out[:].opt()])
                    nc.gpsimd.dma_gather(hR[:, 1], b_out[:],
                                         remidx_sb[:], 2 * P, 2 * P, 512)

                oaccB = new_oacc()
                attn_sts(l, 0, 1, oaccB, range(0, 4), 0, 3,
                         fillers=[ln1w1_coll,
                                  lambda: kv_local(l, 1, on_act=True),
                                  lambda: q_mats(l, 1)])
                part01 = spill(1, oaccB)

                kvr_w0, kvr_w1 = kv_remote_pieces(l)
                oaccA = new_oacc()
                attn_sts(l, 0, 0, oaccA, range(4, 8), 4, 7, fillers=kvr_w0)
                part00 = spill(0, oaccA, part00)

                oaccB = new_oacc()
                attn_sts(l, 0, 1, oaccB, range(4, 8), 4, 7)
                part01 = spill(1, oaccB, part01)

                # ---- tcn0: remote s-tiles (w0-remote first -- only needs
                # collective #1); the w1-remote K/V are emitted between the
                # two halves so a late gather #2 can't block the st8-11 S's.
                oaccC = new_oacc()
                attn_sts(l, 0, 0, oaccC, range(8, 12), 8, 15)
                for t in kvr_w1:
                    t()
                attn_sts(l, 0, 0, oaccC, range(12, 16), 8, 15)
                epi00 = epilogue(0, 0, oaccC, part=part00)

                oaccD = new_oacc()
                attn_sts(l, 0, 1, oaccD, range(8, 16), 8, 15, fillers=epi00)
                epi01 = epilogue(0, 1, oaccD, part=part01)

                # ---- tcn1: single-span windows with tail work as fillers,
                # spaced with no-ops so mid-stream Act ops (LN2's Ln/Exp) get
                # their dependency chains resolved before Act reaches them.
                noop = lambda: None
                pf0 = proj_ffn_thunks(l, 0)
                oaccE = new_oacc()
                attn_sts(l, 1, 0, oaccE, range(0, 16), 0, 15,
                         fillers=epi01 + pf0[:2] + [noop] * 4 + pf0[2:])

                oaccF = new_oacc()
                fill = epilogue(1, 0, oaccE) + [noop]
                if l + 1 < L:
                    fill = fill + prologue_thunks(l + 1)
                else:
                    fill = fill + [lambda: lnf_pool(0)]
                attn_sts(l, 1, 1, oaccF, range(0, 16), 0, 15, fillers=fill)

                for t in epilogue(1, 1, oaccF):
                    t()
                for t in proj_ffn_thunks(l, 1):
                    t()

            # ---------------- final LN + pool + classifier ----------------
            # lnf_pool(0) is emitted as a filler inside the last attention
            # window; lnf_pool(1) runs after the last FFN.
            lnf_pool(1)
            emb = sp.tile([P, 2], F32, tag="emb")
            for cc in range(2):
                nc.vector.tensor_add(emb[:, cc:cc + 1], emb4[:, 0, cc:cc + 1],
                                     emb4[:, 1, cc:cc + 1])
            be_in = dp.tile([P, 2], F32, tag="be_in", name="be_in")
            be_out = dp.tile([2 * P, 2], F32, tag="be_out", name="be_out")
            nc.sync.dma_start(be_in[:], emb[:])
            if sim:
                nc.sync.dma_start(be_out[0:P, :], be_in[:])
                nc.sync.dma_start(be_out[P:2 * P, :], be_in[:])
            else:
                nc.gpsimd.collective_compute(
                    "AllGather", Alu.bypass, replica_groups=REPL,
                    ins=[be_in[:].opt()], outs=[be_out[:].opt()])
            embg = sp.tile([P, 2, 2], F32, tag="embg")
            nc.sync.dma_start(embg[:, 0, :], be_out[0:P, :])
            nc.sync.dma_start(embg[:, 1, :], be_out[P:2 * P, :])
            embr = sp.tile([P, 2], F32, tag="embr")
            nc.vector.tensor_add(embr[:], embg[:, 0, :], embg[:, 1, :])

            h1ps = psM.tile([P, CLS_H // P], F32, tag="mm", name="h1ps")
            for mt in range(CLS_H // P):
                for kc in range(2):
                    nc.tensor.matmul(h1ps[:, mt:mt + 1],
                                     lhsT=wc1[:, kc, mt * P:(mt + 1) * P],
                                     rhs=embr[:, kc:kc + 1],
                                     start=(kc == 0), stop=(kc == 1))
            h1 = sp.tile([P, CLS_H // P], F32, tag="h1")
            nc.vector.tensor_add(h1[:], h1ps[:], bc1[:])
            nc.vector.tensor_scalar_max(h1[:], h1[:], 0.0)
            lps = psM.tile([1, NOUT], F32, tag="mm", name="lps")
            for j in range(CLS_H // P):
                nc.tensor.matmul(lps[:], lhsT=h1[:, j:j + 1], rhs=wc2[:, j, :],
                                 start=(j == 0), stop=(j == CLS_H // P - 1))
            lsb = sp.tile([1, NOUT], F32, tag="lsb")
            nc.vector.tensor_add(lsb[:], lps[:], bc2[:])
            # logits are O(0.1) here, so the usual max-subtraction before the
            # softmax exp is unnecessary -- saves two serial ops in the tail.
            esb = sp.tile([1, NOUT], F32, tag="esb")
            nc.scalar.activation(esb[:], lsb[:], Act.Exp)
            ssum = sp.tile([1, 1], F32, tag="ssum")
            nc.vector.reduce_sum(ssum[:], esb[:], axis=X_AXIS)
            rsum = sp.tile([1, 1], F32, tag="rsum")
            nc.vector.reciprocal(rsum[:], ssum[:])
            probs = sp.tile([1, NOUT], F32, tag="probs")
            nc.vector.tensor_single_scalar(probs[:], esb[:], rsum[:], Alu.mult)
            nc.sync.dma_start(out_d[:], probs[:])

    nc.compile()
    return nc


def _prep_shared(inputs):
    """Host-side weight prepack (identical for all cores)."""
    f = lambda a: np.ascontiguousarray(np.asarray(a, dtype=np.float32))

    def pack_mat(w):  # [C_in, M] -> [128, C_in//128, M]
        ci, m = w.shape
        return np.ascontiguousarray(w.reshape(ci // P, P, m).transpose(1, 0, 2))

    def bf(a):
        return np.ascontiguousarray(a.astype(NPBF16))

    wq3 = np.stack([pack_mat(f(inputs["Wq"][l]).transpose(1, 0, 2).reshape(C, H * HS))
                    for l in range(L)])
    wk3 = np.stack([pack_mat(f(inputs["Wk"][l]).transpose(1, 0, 2).reshape(C, H * HS))
                    for l in range(L)])
    wv3 = np.stack([pack_mat(f(inputs["Wv"][l]).transpose(1, 0, 2).reshape(C, H * HS))
                    for l in range(L)])
    wp3 = np.stack([pack_mat(f(inputs["Wproj"][l])) for l in range(L)])
    w13 = np.stack([pack_mat(f(inputs["W1"][l])) for l in range(L)])
    w23 = np.stack([pack_mat(f(inputs["W2"][l])) for l in range(L)])

    def pack_vec(v):  # [256] -> [128, 2]
        return np.ascontiguousarray(f(v).reshape(2, P).T)

    vecs = np.stack([np.stack([pack_vec(inputs[k][l]) for k in
                               ("ln1_g", "ln1_b", "ln2_g", "ln2_b",
                                "bproj", "b1", "b2")]).transpose(1, 0, 2)
                     for l in range(L)])
    vecs = np.ascontiguousarray(vecs)
    lnfv = np.ascontiguousarray(
        np.stack([pack_vec(inputs["lnf_g"]),
                  pack_vec(inputs["lnf_b"])]).transpose(1, 0, 2))
    wc1 = pack_mat(f(inputs["Wc1"]) / T)        # fold mean-pool 1/T into Wc1
    bc1 = np.ascontiguousarray(f(inputs["bc1"]).reshape(CLS_H // P, P).T)
    wc2 = np.ascontiguousarray(f(inputs["Wc2"]).reshape(CLS_H // P, P, NOUT)
                               .transpose(1, 0, 2))
    bc2 = f(inputs["bc2"]).reshape(1, NOUT)
    tokf = f(inputs["tok_emb"])
    posf = f(inputs["pos_emb"])
    return dict(wq=bf(wq3), wk=bf(wk3), wv=bf(wv3), wp=bf(wp3), w1=bf(w13),
                w2=bf(w23), vecs=vecs, lnf=lnfv, wc1=wc1, bc1=bc1, wc2=wc2,
                bc2=bc2, tok=tokf, pos=posf)


def _wrap_idx(ids):
    """int array [n] -> dma_gather wrapped layout [128, n//16] int16."""
    n = ids.shape[0]
    w = ids.reshape(n // 16, 16).T.astype(np.int16)     # [16, n//16]
    return np.ascontiguousarray(np.tile(w, (8, 1)))     # [128, n//16]


def _make_in_maps(inputs):
    shared = _prep_shared(inputs)
    idx = np.asarray(inputs["idx"]).astype(np.int64)
    in_maps = []
    for c in range(N_CORES):
        b, th = c // 2, c % 2
        t0 = th * TL
        idx_loc = idx[b, t0:t0 + TL]
        pos_loc = shared["pos"][t0:t0 + TL]  # [TL, C]
        posr_a = np.ascontiguousarray(
            pos_loc.reshape(TL // P, P, C).transpose(1, 0, 2))
        rem = (1 - th) * 2 * P + np.arange(2 * P, dtype=np.int64)
        m = dict(tok=shared["tok"], idxw=_wrap_idx(idx_loc), posr=posr_a,
                 remidx=_wrap_idx(rem),
                 wq=shared["wq"], wk=shared["wk"], wv=shared["wv"],
                 wp=shared["wp"], w1=shared["w1"], w2=shared["w2"],
                 vecs=shared["vecs"], lnf=shared["lnf"], wc1=shared["wc1"],
                 bc1=shared["bc1"], wc2=shared["wc2"], bc2=shared["bc2"])
        in_maps.append(m)
    return in_maps


def kernel(**inputs) -> np.ndarray:
    if "nc" not in _CACHE:
        _CACHE["nc"] = _build_program()
    nc = _CACHE["nc"]
    in_maps = _make_in_maps(inputs)
    res = bass_utils.run_bass_kernel_spmd(nc, in_maps, core_ids=list(range(N_CORES)))
    out = np.zeros((B, NOUT), np.float32)
    for b in range(B):
        out[b] = res.results[2 * b]["probs"][0]
    return out
